# revision 11
# baseline (speedup 1.0000x reference)
"""BSplineKAN layer kernel for 8 Trainium2 NeuronCores.

Math
----
The reference computes, per element x = clip(x, -1, 1):
    y[n,o] = sum_{i,b} basis_b(x[n,i]) * coeff[o,i,b]  +  silu(x) @ w_base.T + bias
where basis is the 7-function clamped cubic B-spline basis on knots
{-1(x4), -0.5, 0, 0.5, 1(x4)}.  A quirk of the reference recurrence: at
x == 1.0 exactly (all clamped x >= 1 inputs) the basis row is all ZERO.

On [-1, 1) the 7 basis functions are C^2 piecewise cubics with breakpoints
at {-0.5, 0, +0.5}; the 7-dim space they span has the center-anchored
truncated-power basis
    feats = [m, m*x, m*x^2, m*x^3, m*x_+^3, m*(x-1/2)_+^3, min(x+1/2,0)^3]
with m = (x < 1) the edge mask (all seven vanish at x == 1, reproducing
the reference's edge behavior exactly).  basis_b = T[f,b] @ feats with T
integer/6, exact.  T is folded into coeff on the host.  silu(x)
is itself a smooth function on [-1,1], so instead of an extra feature it
is least-squares-fit in the SAME 7-dim spline space (max resid 5.3e-5)
and folded into the weights; the x==1 edge (features vanish, silu(1)
=0.731) folds exactly as 0.731*(1-m): the constant part goes to the
bias, the -0.731*m part into the f0 weight row.  One fused fp16 matmul
    y[n,o] = sum_{i,f} F_f(x[n,i]) * W[f,i,o] + bias'
with K = 7*1024 = 7168 (vs 11*1024 for the two-half-window local basis
with an explicit silu feature: 36% fewer FLOPs for a ~3x larger - still
~2e-3, 10x under the gate - fp16 cancellation error).
fp16 operands with fp32 PSUM accumulation.

x ships as fp16, pre-clamped on the host.  Values in (1-2^-12, 1) that
fp16 would round to exactly 1.0 are pinned to 1-2^-11 so the on-chip
mask m = (x < 1) matches the reference's fp32 comparison; this halves
the x DMA and removes the on-chip clamp from every dependency chain.

Distribution: 4-way batch x 2-way d_out mesh over 8 cores.  Per core:
x arrives host-transposed as (1024, 2048) fp16, W-shard (8192, 512)
fp16 stays resident in SBUF, output (2048, 512) fp32.  Features are
computed on DVE/ACT/Pool, and TensorE runs back-to-back 64-tile
K-accumulations into PSUM at the fp16 roofline (~213 ns per
512-column matmul).  Startup DMA order + PE warm-up are tuned so the
first real matmul issues ~6 us in at full clock with zero PE idle
thereafter; evictions are emitted after the next chunk's feature ops
so they never block a chunk boundary, and the final eviction is
pipelined in two column pieces to shorten the drain tail.
"""

import numpy as np

# ---- problem constants (hardcoded per contract) ----
N_FULL, D_IN, D_OUT = 8192, 1024, 1024
MESH_N, MESH_O = 4, 2                 # 4-way batch x 2-way d_out
N_SHARD = N_FULL // MESH_N            # 2048
O_SHARD = D_OUT // MESH_O             # 512
P = 128
NF = 7                                # 7 spline features (silu folded in)
IB = D_IN // P                        # 8 i-blocks
KT = IB * NF                          # 56 K-tiles
NCHUNK = 256                          # batch cols per pipeline chunk
NSUB = NCHUNK // P                    # 2
CHUNKS = N_SHARD // NCHUNK            # 8
N_WARM = 72                           # PE warm-up matmuls (p-state + DMA cover)

# basis_b = sum_f feats_f * T6[f, b] / 6; feats order:
# [m, m*x, m*x^2, m*x^3, m*relu(x)^3, m*relu(x-1/2)^3, min(x+1/2,0)^3]
_T6 = np.array([
    [0,    0,    1,    4,    1,    0,   0],
    [0,    0,   -6,    0,    6,    0,   0],
    [0,    0,   12,  -24,   12,    0,   0],
    [0,  -12,   28,  -24,    8,    0,   0],
    [0,   12,  -36,   48,  -36,   12,   0],
    [0,    0,    8,  -32,   72,  -96,  48],
    [-48, 96,  -72,   32,   -8,    0,   0],
], dtype=np.float64)

# silu(x) ~= sum_f SILU_FIT[f] * feats_f(x) on [-1, 1)  (max resid 5.3e-5)
_SILU_FIT = np.array([-5.30336056e-05, 5.00000000e-01, 2.55431861e-01,
                      2.08452191e-02, -4.16904381e-02, -2.79657411e-02,
                      2.79657403e-02], dtype=np.float64)
_SILU_AT_1 = 0.7310585786300049

_PROGRAM = None  # compiled Bass program, built once


def _build_program():
    import concourse.mybir as mybir
    import concourse.tile as tile
    from concourse import bacc

    f32 = mybir.dt.float32
    f16 = mybir.dt.float16
    Op = mybir.AluOpType

    nc = bacc.Bacc("TRN2", target_bir_lowering=False, debug=False)
    xt_d = nc.dram_tensor("xt", [D_IN, N_SHARD], f16, kind="ExternalInput").ap()
    w_d = nc.dram_tensor("wt", [KT * P, O_SHARD], f16, kind="ExternalInput").ap()
    b_d = nc.dram_tensor("biasb", [P, O_SHARD], f32, kind="ExternalInput").ap()
    y_d = nc.dram_tensor("y", [N_SHARD, O_SHARD], f32, kind="ExternalOutput").ap()

    with tile.TileContext(nc) as tc:
        with (
            tc.tile_pool(name="const", bufs=1) as const_pool,
            tc.tile_pool(name="wt", bufs=1) as wt_pool,
            tc.tile_pool(name="feat", bufs=2) as f_pool,
            tc.tile_pool(name="xc", bufs=2) as xc_pool,
            tc.tile_pool(name="tmp", bufs=2) as tmp_pool,
            tc.tile_pool(name="out", bufs=3) as out_pool,
            tc.tile_pool(name="pso", bufs=4, space="PSUM") as psum_out,
        ):
            # Startup DMAs are emitted FIRST so the sync engine issues them
            # immediately.  Each chunk-0 x i-block piece rides just before
            # its weight slab: the stream supplies a slab every ~2.7us while
            # the PE consumes one every ~3.0us, so after slab 0 lands the PE
            # never waits.  Chunk-1 x and the bias follow the last slab.
            xt_r = xt_d.rearrange("(ib p) n -> p ib n", p=P)
            xc0 = xc_pool.tile([P, IB, NCHUNK], f16, tag="xc", name="xc0")
            xc1 = xc_pool.tile([P, IB, NCHUNK], f16, tag="xc", name="xc1")

            wt = {}
            def load_wt(ib):
                t = wt_pool.tile([P, NF, O_SHARD], f16, tag=f"wt_{ib}", name=f"wt_{ib}")
                r0 = ib * NF * P
                nc.sync.dma_start(
                    t[:], w_d[r0:r0 + NF * P, :].rearrange("(f p) o -> p f o", p=P))
                wt[ib] = t
            for ib in range(IB - 1):
                nc.sync.dma_start(xc0[:, ib:ib + 1], xt_r[:, ib:ib + 1, 0:NCHUNK])
                load_wt(ib)
            nc.sync.dma_start(xc0[:, IB - 1:], xt_r[:, IB - 1:, 0:NCHUNK])
            # chunk-1 x rides before the last slab (chunk-1's feature chain
            # needs a ~2.5us head start on the boundary); slab 7 is split so
            # its first half still arrives before the PE reaches i-block 7
            nc.sync.dma_start(xc1[:, :1], xt_r[:, :1, NCHUNK:2 * NCHUNK])
            nc.sync.dma_start(xc1[:, 1:], xt_r[:, 1:, NCHUNK:2 * NCHUNK])
            ib7 = IB - 1
            wt7 = wt_pool.tile([P, NF, O_SHARD], f16, tag=f"wt_{ib7}",
                               name=f"wt_{ib7}")
            r7 = ib7 * NF * P
            nc.sync.dma_start(
                wt7[:, :4],
                w_d[r7:r7 + 4 * P, :].rearrange("(f p) o -> p f o", p=P))
            nc.sync.dma_start(
                wt7[:, 4:],
                w_d[r7 + 4 * P:r7 + NF * P, :].rearrange("(f p) o -> p f o", p=P))
            wt[ib7] = wt7
            bias_s = const_pool.tile([P, O_SHARD], f32)
            nc.sync.dma_start(bias_s[:], b_d[:])

            # PE warm-up: burns the p-state ramp while the startup DMAs
            # stream, so real matmuls run at full clock from the start.
            wz = const_pool.tile([P, P], f16, name="wz")
            nc.gpsimd.memset(wz[:], 0.0)
            pw = psum_out.tile([P, 64], f32, tag="pwarm", name="pwarm")
            for i in range(N_WARM):
                nc.tensor.matmul(pw[:], wz[:], wz[:, :64],
                                 start=(i == 0), stop=(i == N_WARM - 1))

            F = {}

            def features(chunk, xch):
                """Emit feature ops for all i-blocks of one chunk."""
                N = NCHUNK
                for ib in range(IB):
                    xcb = xch[:, ib]

                    def single(f):
                        t = f_pool.tile([P, NCHUNK], f16, tag=f"F_{ib}_{f}",
                                        name=f"F_{ib}_{f}")
                        F[ib, f] = t
                        return t

                    # mask m = (x < 1), exact 0/1 in fp16 (Pool)
                    m = single(0)
                    nc.gpsimd.tensor_scalar(m[:], xcb[:], 1.0, None, Op.is_lt)
                    # A = [x-1/2 | x+1/2], A2 = A*A, A3 = A2*A (all DVE fp16;
                    # one in-order queue -> no cross-engine latency on the
                    # chunk-boundary critical chain, and ACT stays empty)
                    A = tmp_pool.tile([P, 2 * NCHUNK], f16, tag="A", name="A")
                    nc.vector.tensor_scalar(A[:, :N], xcb[:], -0.5, None, Op.add)
                    nc.vector.tensor_scalar(A[:, N:], xcb[:], 0.5, None, Op.add)
                    A2 = tmp_pool.tile([P, 2 * NCHUNK], f16, tag="A2", name="A2")
                    nc.vector.tensor_tensor(A2[:], A[:], A[:], Op.mult)
                    A3 = tmp_pool.tile([P, 2 * NCHUNK], f16, tag="A3", name="A3")
                    nc.vector.tensor_tensor(A3[:], A2[:], A[:], Op.mult)
                    # f5 = m*relu((x-1/2)^3); f6 = min(x+1/2,0)^3 (self-masked)
                    nc.vector.scalar_tensor_tensor(single(5)[:], A3[:, :N], 0.0,
                                                   m[:], Op.max, Op.mult)
                    nc.gpsimd.tensor_scalar(single(6)[:], A3[:, N:], 0.0, None,
                                            Op.min)
                    # monomial chain: f1 = m*x, f2 = f1^2, f3 = f1*f2,
                    # f4 = relu(f3)
                    f1 = single(1)
                    nc.vector.tensor_tensor(f1[:], m[:], xcb[:], Op.mult)
                    f2 = single(2)
                    nc.vector.tensor_tensor(f2[:], f1[:], f1[:], Op.mult)
                    f3 = single(3)
                    nc.vector.tensor_tensor(f3[:], f1[:], f2[:], Op.mult)
                    nc.gpsimd.tensor_scalar(single(4)[:], f3[:], 0.0, None, Op.max)

            def lhs(ib, f, ns, Fc):
                return Fc[ib, f][:, ns * P:(ns + 1) * P]

            def evict(chunk, ps, ns, pieces=1):
                c0 = chunk * NCHUNK
                r0 = c0 + ns * P
                W = O_SHARD // pieces
                o = out_pool.tile([P, O_SHARD], f32, tag="out", name="outt")
                for j in range(pieces):
                    nc.vector.tensor_tensor(o[:, j * W:(j + 1) * W],
                                            ps[:, j * W:(j + 1) * W],
                                            bias_s[:, j * W:(j + 1) * W], Op.add)
                    nc.sync.dma_start(y_d[r0:r0 + P, j * W:(j + 1) * W],
                                      o[:, j * W:(j + 1) * W])

            # chunk 0 features (ib0's depend only on the tiny first x DMA)
            features(0, xc0)
            Fprev = dict(F)

            pending = []   # evictions deferred past the next chunk's features
            for chunk in range(CHUNKS):
                Fc = Fprev
                # -- matmuls. Chunk 0 runs k-major over both 128-batch
                # subtiles so each weight slab feeds two matmuls the moment
                # its DMA lands; later chunks run the subtiles serially so
                # group-0's eviction overlaps group-1's matmuls --
                if chunk == 0:
                    pss = [psum_out.tile([P, O_SHARD], f32, tag=f"psout{ns}",
                                         name=f"psout{ns}", bufs=2)
                           for ns in range(NSUB)]
                    for k, (ib, f) in enumerate(
                            (ib, f) for ib in range(IB) for f in range(NF)):
                        for ns in range(NSUB):
                            nc.tensor.matmul(
                                pss[ns][:], lhs(ib, f, ns, Fc), wt[ib][:, f],
                                start=(k == 0), stop=(k == KT - 1))
                    pending = [(chunk, pss[0], 0), (chunk, pss[1], 1)]
                else:
                    for ns in range(NSUB):
                        ps = psum_out.tile([P, O_SHARD], f32, tag=f"psout{ns}",
                                           name=f"psout{ns}", bufs=2)
                        for k, (ib, f) in enumerate(
                                (ib, f) for ib in range(IB) for f in range(NF)):
                            nc.tensor.matmul(
                                ps[:], lhs(ib, f, ns, Fc), wt[ib][:, f],
                                start=(k == 0), stop=(k == KT - 1))
                        if ns == 0:
                            # mid-chunk psum: evict immediately, overlaps ns1
                            evict(chunk, ps, 0)
                        else:
                            pending.append((chunk, ps, 1))

                # next chunk's x DMA + features BEFORE the deferred
                # evictions so the boundary dependency chain (A3/f1/f3 on
                # DVE) is not queued behind a psum wait
                if chunk + 1 < CHUNKS:
                    if chunk + 1 == 1:
                        xch = xc1
                    else:
                        c1 = (chunk + 1) * NCHUNK
                        xch = xc_pool.tile([P, IB, NCHUNK], f16, tag="xc",
                                           name="xc")
                        nc.sync.dma_start(xch[:], xt_r[:, :, c1:c1 + NCHUNK])
                    F = {}
                    features(chunk + 1, xch)
                    Fprev = dict(F)
                for (ec, eps, ens) in pending:
                    # last eviction of the run: pipeline DVE + DMA in pieces
                    last = (chunk == CHUNKS - 1)
                    evict(ec, eps, ens, pieces=2 if last else 1)
                pending = []

    nc.compile()
    return nc


def _fold_weights(coeff, w_base):
    """Fold the feature->basis matrix into coeff and absorb the silu/w_base
    path into the same 7 feature rows; returns ((K, D_OUT) fp16, bias_add)."""
    T = _T6 / 6.0
    c64 = np.asarray(coeff).astype(np.float64)
    wb = np.asarray(w_base).astype(np.float64)
    # Wf[f, i, o] = sum_b T[f, b] * coeff[o, i, b]  (+ silu fit via w_base)
    Wf = np.einsum('fb,oib->fio', T, c64)
    Wf += _SILU_FIT[:, None, None] * wb.T[None]
    Wf[0] -= _SILU_AT_1 * wb.T          # silu(1)*(1-m): -m part
    bias_add = _SILU_AT_1 * wb.sum(axis=1)   # constant part -> bias
    # pack K as (ib, f, p): row k = ib*(NF*P) + f*P + p  <->  Wf[f, ib*P+p, o]
    Wt = Wf.reshape(NF, IB, P, D_OUT).transpose(1, 0, 2, 3).reshape(KT * P, D_OUT)
    return Wt.astype(np.float16), bias_add


def _prep_x16(x):
    """Host-side clamp to [-1,1] in fp16 with exact mask semantics at +1:
    any x < 1 that fp16 would round to 1.0 is pinned one ulp below."""
    x = np.asarray(x, dtype=np.float32)
    x16 = np.clip(x, -1.0, 1.0).astype(np.float16)
    edge = np.float16(1.0 - 2.0 ** -11)
    fix = (x < 1.0) & (x16 >= 1.0)
    if fix.any():
        x16[fix] = edge
    return x16


def kernel(x, coeff, w_base, bias):
    global _PROGRAM
    from concourse.bass_utils import run_bass_kernel_spmd

    if _PROGRAM is None:
        _PROGRAM = _build_program()
    nc = _PROGRAM

    x16 = _prep_x16(x)
    Wt, bias_add = _fold_weights(coeff, w_base)
    bias = (np.asarray(bias, dtype=np.float64) + bias_add).astype(np.float32)

    in_maps = []
    for core in range(8):
        cn, co = divmod(core, MESH_O)
        in_maps.append({
            "xt": np.ascontiguousarray(x16[cn * N_SHARD:(cn + 1) * N_SHARD].T),
            "wt": np.ascontiguousarray(Wt[:, co * O_SHARD:(co + 1) * O_SHARD]),
            "biasb": np.ascontiguousarray(np.broadcast_to(
                bias[co * O_SHARD:(co + 1) * O_SHARD], (P, O_SHARD)).astype(np.float32)),
        })

    res = run_bass_kernel_spmd(nc, in_maps, list(range(8)))

    y = np.empty((N_FULL, D_OUT), dtype=np.float32)
    for core in range(8):
        cn, co = divmod(core, MESH_O)
        y[cn * N_SHARD:(cn + 1) * N_SHARD, co * O_SHARD:(co + 1) * O_SHARD] = \
            res.results[core]["y"]
    return y


# revision 12
# speedup vs baseline: 1.0002x; 1.0002x over previous
"""BSplineKAN layer kernel for 8 Trainium2 NeuronCores.

Math
----
The reference computes, per element x = clip(x, -1, 1):
    y[n,o] = sum_{i,b} basis_b(x[n,i]) * coeff[o,i,b]  +  silu(x) @ w_base.T + bias
where basis is the 7-function clamped cubic B-spline basis on knots
{-1(x4), -0.5, 0, 0.5, 1(x4)}.  A quirk of the reference recurrence: at
x == 1.0 exactly (all clamped x >= 1 inputs) the basis row is all ZERO.

On [-1, 1) the 7 basis functions are C^2 piecewise cubics with breakpoints
at {-0.5, 0, +0.5}; the 7-dim space they span has the center-anchored
truncated-power basis
    feats = [m, m*x, m*x^2, m*x^3, m*x_+^3, m*(x-1/2)_+^3, min(x+1/2,0)^3]
with m = (x < 1) the edge mask (all seven vanish at x == 1, reproducing
the reference's edge behavior exactly).  basis_b = T[f,b] @ feats with T
integer/6, exact.  T is folded into coeff on the host.  silu(x)
is itself a smooth function on [-1,1], so instead of an extra feature it
is least-squares-fit in the SAME 7-dim spline space (max resid 5.3e-5)
and folded into the weights; the x==1 edge (features vanish, silu(1)
=0.731) folds exactly as 0.731*(1-m): the constant part goes to the
bias, the -0.731*m part into the f0 weight row.  One fused fp16 matmul
    y[n,o] = sum_{i,f} F_f(x[n,i]) * W[f,i,o] + bias'
with K = 7*1024 = 7168 (vs 11*1024 for the two-half-window local basis
with an explicit silu feature: 36% fewer FLOPs for a ~3x larger - still
~2e-3, 10x under the gate - fp16 cancellation error).
fp16 operands with fp32 PSUM accumulation.

x ships as fp16, pre-clamped on the host.  Values in (1-2^-12, 1) that
fp16 would round to exactly 1.0 are pinned to 1-2^-11 so the on-chip
mask m = (x < 1) matches the reference's fp32 comparison; this halves
the x DMA and removes the on-chip clamp from every dependency chain.

Distribution: 4-way batch x 2-way d_out mesh over 8 cores.  Per core:
x arrives host-transposed as (1024, 2048) fp16, W-shard (8192, 512)
fp16 stays resident in SBUF, output (2048, 512) fp32.  Features are
computed on DVE/ACT/Pool, and TensorE runs back-to-back 64-tile
K-accumulations into PSUM at the fp16 roofline (~213 ns per
512-column matmul).  Startup DMA order + PE warm-up are tuned so the
first real matmul issues ~6 us in at full clock with zero PE idle
thereafter; evictions are emitted after the next chunk's feature ops
so they never block a chunk boundary, and the final eviction is
pipelined in two column pieces to shorten the drain tail.
"""

import numpy as np

# ---- problem constants (hardcoded per contract) ----
N_FULL, D_IN, D_OUT = 8192, 1024, 1024
MESH_N, MESH_O = 4, 2                 # 4-way batch x 2-way d_out
N_SHARD = N_FULL // MESH_N            # 2048
O_SHARD = D_OUT // MESH_O             # 512
P = 128
NF = 7                                # 7 spline features (silu folded in)
IB = D_IN // P                        # 8 i-blocks
KT = IB * NF                          # 56 K-tiles
NCHUNK = 256                          # batch cols per pipeline chunk
NSUB = NCHUNK // P                    # 2
CHUNKS = N_SHARD // NCHUNK            # 8
N_WARM = 72                           # PE warm-up matmuls (p-state + DMA cover)

# basis_b = sum_f feats_f * T6[f, b] / 6; feats order:
# [m, m*x, m*x^2, m*x^3, m*relu(x)^3, m*relu(x-1/2)^3, min(x+1/2,0)^3]
_T6 = np.array([
    [0,    0,    1,    4,    1,    0,   0],
    [0,    0,   -6,    0,    6,    0,   0],
    [0,    0,   12,  -24,   12,    0,   0],
    [0,  -12,   28,  -24,    8,    0,   0],
    [0,   12,  -36,   48,  -36,   12,   0],
    [0,    0,    8,  -32,   72,  -96,  48],
    [-48, 96,  -72,   32,   -8,    0,   0],
], dtype=np.float64)

# silu(x) ~= sum_f SILU_FIT[f] * feats_f(x) on [-1, 1)  (max resid 5.3e-5)
_SILU_FIT = np.array([-5.30336056e-05, 5.00000000e-01, 2.55431861e-01,
                      2.08452191e-02, -4.16904381e-02, -2.79657411e-02,
                      2.79657403e-02], dtype=np.float64)
_SILU_AT_1 = 0.7310585786300049

_PROGRAM = None  # compiled Bass program, built once


def _build_program():
    import concourse.mybir as mybir
    import concourse.tile as tile
    from concourse import bacc

    f32 = mybir.dt.float32
    f16 = mybir.dt.float16
    Op = mybir.AluOpType

    nc = bacc.Bacc("TRN2", target_bir_lowering=False, debug=False)
    xt_d = nc.dram_tensor("xt", [D_IN, N_SHARD], f16, kind="ExternalInput").ap()
    w_d = nc.dram_tensor("wt", [KT * P, O_SHARD], f16, kind="ExternalInput").ap()
    b_d = nc.dram_tensor("biasb", [P, O_SHARD], f32, kind="ExternalInput").ap()
    y_d = nc.dram_tensor("y", [N_SHARD, O_SHARD], f32, kind="ExternalOutput").ap()

    with tile.TileContext(nc) as tc:
        with (
            tc.tile_pool(name="const", bufs=1) as const_pool,
            tc.tile_pool(name="wt", bufs=1) as wt_pool,
            tc.tile_pool(name="feat", bufs=2) as f_pool,
            tc.tile_pool(name="xc", bufs=2) as xc_pool,
            tc.tile_pool(name="tmp", bufs=2) as tmp_pool,
            tc.tile_pool(name="out", bufs=3) as out_pool,
            tc.tile_pool(name="pso", bufs=4, space="PSUM") as psum_out,
        ):
            # Startup DMAs are emitted FIRST so the sync engine issues them
            # immediately.  Each chunk-0 x i-block piece rides just before
            # its weight slab: the stream supplies a slab every ~2.7us while
            # the PE consumes one every ~3.0us, so after slab 0 lands the PE
            # never waits.  Chunk-1 x and the bias follow the last slab.
            xt_r = xt_d.rearrange("(ib p) n -> p ib n", p=P)
            xc0 = xc_pool.tile([P, IB, NCHUNK], f16, tag="xc", name="xc0")
            xc1 = xc_pool.tile([P, IB, NCHUNK], f16, tag="xc", name="xc1")

            wt = {}
            def load_wt(ib):
                t = wt_pool.tile([P, NF, O_SHARD], f16, tag=f"wt_{ib}", name=f"wt_{ib}")
                r0 = ib * NF * P
                nc.sync.dma_start(
                    t[:], w_d[r0:r0 + NF * P, :].rearrange("(f p) o -> p f o", p=P))
                wt[ib] = t
            for ib in range(IB - 1):
                nc.sync.dma_start(xc0[:, ib:ib + 1], xt_r[:, ib:ib + 1, 0:NCHUNK])
                load_wt(ib)
            nc.sync.dma_start(xc0[:, IB - 1:], xt_r[:, IB - 1:, 0:NCHUNK])
            # chunk-1 x rides before the last slab (chunk-1's feature chain
            # needs a ~2.5us head start on the boundary); slab 7 is split so
            # its first half still arrives before the PE reaches i-block 7
            nc.sync.dma_start(xc1[:, :1], xt_r[:, :1, NCHUNK:2 * NCHUNK])
            nc.sync.dma_start(xc1[:, 1:], xt_r[:, 1:, NCHUNK:2 * NCHUNK])
            ib7 = IB - 1
            wt7 = wt_pool.tile([P, NF, O_SHARD], f16, tag=f"wt_{ib7}",
                               name=f"wt_{ib7}")
            r7 = ib7 * NF * P
            nc.sync.dma_start(
                wt7[:, :4],
                w_d[r7:r7 + 4 * P, :].rearrange("(f p) o -> p f o", p=P))
            nc.sync.dma_start(
                wt7[:, 4:],
                w_d[r7 + 4 * P:r7 + NF * P, :].rearrange("(f p) o -> p f o", p=P))
            wt[ib7] = wt7
            bias_s = const_pool.tile([P, O_SHARD], f32)
            nc.sync.dma_start(bias_s[:], b_d[:])

            # PE warm-up: burns the p-state ramp while the startup DMAs
            # stream, so real matmuls run at full clock from the start.
            wz = const_pool.tile([P, P], f16, name="wz")
            nc.gpsimd.memset(wz[:], 0.0)
            pw = psum_out.tile([P, 64], f32, tag="pwarm", name="pwarm")
            for i in range(N_WARM):
                nc.tensor.matmul(pw[:], wz[:], wz[:, :64],
                                 start=(i == 0), stop=(i == N_WARM - 1))

            F = {}

            def features(chunk, xch):
                """Emit feature ops for all i-blocks of one chunk."""
                N = NCHUNK
                for ib in range(IB):
                    xcb = xch[:, ib]

                    def single(f):
                        t = f_pool.tile([P, NCHUNK], f16, tag=f"F_{ib}_{f}",
                                        name=f"F_{ib}_{f}")
                        F[ib, f] = t
                        return t

                    # mask m = (x < 1), exact 0/1 in fp16 (Pool)
                    m = single(0)
                    nc.gpsimd.tensor_scalar(m[:], xcb[:], 1.0, None, Op.is_lt)
                    # A = [x-1/2 | x+1/2], A2 = A*A, A3 = A2*A (all DVE fp16;
                    # one in-order queue -> no cross-engine latency on the
                    # chunk-boundary critical chain, and ACT stays empty)
                    A = tmp_pool.tile([P, 2 * NCHUNK], f16, tag="A", name="A")
                    nc.vector.tensor_scalar(A[:, :N], xcb[:], -0.5, None, Op.add)
                    nc.vector.tensor_scalar(A[:, N:], xcb[:], 0.5, None, Op.add)
                    A2 = tmp_pool.tile([P, 2 * NCHUNK], f16, tag="A2", name="A2")
                    nc.vector.tensor_tensor(A2[:], A[:], A[:], Op.mult)
                    A3 = tmp_pool.tile([P, 2 * NCHUNK], f16, tag="A3", name="A3")
                    nc.vector.tensor_tensor(A3[:], A2[:], A[:], Op.mult)
                    # f5 = m*relu((x-1/2)^3); f6 = min(x+1/2,0)^3 (self-masked)
                    nc.vector.scalar_tensor_tensor(single(5)[:], A3[:, :N], 0.0,
                                                   m[:], Op.max, Op.mult)
                    nc.gpsimd.tensor_scalar(single(6)[:], A3[:, N:], 0.0, None,
                                            Op.min)
                    # monomial chain: f1 = m*x, f2 = f1^2, f3 = f1*f2,
                    # f4 = relu(f3)
                    f1 = single(1)
                    nc.vector.tensor_tensor(f1[:], m[:], xcb[:], Op.mult)
                    f2 = single(2)
                    nc.vector.tensor_tensor(f2[:], f1[:], f1[:], Op.mult)
                    f3 = single(3)
                    nc.vector.tensor_tensor(f3[:], f1[:], f2[:], Op.mult)
                    nc.gpsimd.tensor_scalar(single(4)[:], f3[:], 0.0, None, Op.max)

            def lhs(ib, f, ns, Fc):
                return Fc[ib, f][:, ns * P:(ns + 1) * P]

            def evict(chunk, ps, ns, widths=(O_SHARD,)):
                c0 = chunk * NCHUNK
                r0 = c0 + ns * P
                o = out_pool.tile([P, O_SHARD], f32, tag="out", name="outt")
                j = 0
                for w in widths:
                    nc.vector.tensor_tensor(o[:, j:j + w], ps[:, j:j + w],
                                            bias_s[:, j:j + w], Op.add)
                    nc.sync.dma_start(y_d[r0:r0 + P, j:j + w], o[:, j:j + w])
                    j += w

            # chunk 0 features (ib0's depend only on the tiny first x DMA)
            features(0, xc0)
            Fprev = dict(F)

            pending = []   # evictions deferred past the next chunk's features
            for chunk in range(CHUNKS):
                Fc = Fprev
                # -- matmuls. Chunk 0 runs k-major over both 128-batch
                # subtiles so each weight slab feeds two matmuls the moment
                # its DMA lands; later chunks run the subtiles serially so
                # group-0's eviction overlaps group-1's matmuls --
                if chunk == 0:
                    pss = [psum_out.tile([P, O_SHARD], f32, tag=f"psout{ns}",
                                         name=f"psout{ns}", bufs=2)
                           for ns in range(NSUB)]
                    for k, (ib, f) in enumerate(
                            (ib, f) for ib in range(IB) for f in range(NF)):
                        for ns in range(NSUB):
                            nc.tensor.matmul(
                                pss[ns][:], lhs(ib, f, ns, Fc), wt[ib][:, f],
                                start=(k == 0), stop=(k == KT - 1))
                    pending = [(chunk, pss[0], 0), (chunk, pss[1], 1)]
                else:
                    for ns in range(NSUB):
                        ps = psum_out.tile([P, O_SHARD], f32, tag=f"psout{ns}",
                                           name=f"psout{ns}", bufs=2)
                        for k, (ib, f) in enumerate(
                                (ib, f) for ib in range(IB) for f in range(NF)):
                            nc.tensor.matmul(
                                ps[:], lhs(ib, f, ns, Fc), wt[ib][:, f],
                                start=(k == 0), stop=(k == KT - 1))
                        if ns == 0:
                            # mid-chunk psum: evict immediately, overlaps ns1
                            evict(chunk, ps, 0)
                        else:
                            pending.append((chunk, ps, 1))

                # next chunk's x DMA + features BEFORE the deferred
                # evictions so the boundary dependency chain (A3/f1/f3 on
                # DVE) is not queued behind a psum wait
                if chunk + 1 < CHUNKS:
                    if chunk + 1 == 1:
                        xch = xc1
                    else:
                        c1 = (chunk + 1) * NCHUNK
                        xch = xc_pool.tile([P, IB, NCHUNK], f16, tag="xc",
                                           name="xc")
                        nc.sync.dma_start(xch[:], xt_r[:, :, c1:c1 + NCHUNK])
                    F = {}
                    features(chunk + 1, xch)
                    Fprev = dict(F)
                for (ec, eps, ens) in pending:
                    # last eviction of the run: pipeline DVE + DMA, with a
                    # small final piece so the tail's fixed per-DMA costs
                    # (desc-gen + dge delay + sem prop) trail a short chain
                    last = (chunk == CHUNKS - 1)
                    evict(ec, eps, ens, widths=(384, 128) if last else (O_SHARD,))
                pending = []

    nc.compile()
    return nc


def _fold_weights(coeff, w_base):
    """Fold the feature->basis matrix into coeff and absorb the silu/w_base
    path into the same 7 feature rows; returns ((K, D_OUT) fp16, bias_add)."""
    T = _T6 / 6.0
    c64 = np.asarray(coeff).astype(np.float64)
    wb = np.asarray(w_base).astype(np.float64)
    # Wf[f, i, o] = sum_b T[f, b] * coeff[o, i, b]  (+ silu fit via w_base)
    Wf = np.einsum('fb,oib->fio', T, c64)
    Wf += _SILU_FIT[:, None, None] * wb.T[None]
    Wf[0] -= _SILU_AT_1 * wb.T          # silu(1)*(1-m): -m part
    bias_add = _SILU_AT_1 * wb.sum(axis=1)   # constant part -> bias
    # pack K as (ib, f, p): row k = ib*(NF*P) + f*P + p  <->  Wf[f, ib*P+p, o]
    Wt = Wf.reshape(NF, IB, P, D_OUT).transpose(1, 0, 2, 3).reshape(KT * P, D_OUT)
    return Wt.astype(np.float16), bias_add


def _prep_x16(x):
    """Host-side clamp to [-1,1] in fp16 with exact mask semantics at +1:
    any x < 1 that fp16 would round to 1.0 is pinned one ulp below."""
    x = np.asarray(x, dtype=np.float32)
    x16 = np.clip(x, -1.0, 1.0).astype(np.float16)
    edge = np.float16(1.0 - 2.0 ** -11)
    fix = (x < 1.0) & (x16 >= 1.0)
    if fix.any():
        x16[fix] = edge
    return x16


def kernel(x, coeff, w_base, bias):
    global _PROGRAM
    from concourse.bass_utils import run_bass_kernel_spmd

    if _PROGRAM is None:
        _PROGRAM = _build_program()
    nc = _PROGRAM

    x16 = _prep_x16(x)
    Wt, bias_add = _fold_weights(coeff, w_base)
    bias = (np.asarray(bias, dtype=np.float64) + bias_add).astype(np.float32)

    in_maps = []
    for core in range(8):
        cn, co = divmod(core, MESH_O)
        in_maps.append({
            "xt": np.ascontiguousarray(x16[cn * N_SHARD:(cn + 1) * N_SHARD].T),
            "wt": np.ascontiguousarray(Wt[:, co * O_SHARD:(co + 1) * O_SHARD]),
            "biasb": np.ascontiguousarray(np.broadcast_to(
                bias[co * O_SHARD:(co + 1) * O_SHARD], (P, O_SHARD)).astype(np.float32)),
        })

    res = run_bass_kernel_spmd(nc, in_maps, list(range(8)))

    y = np.empty((N_FULL, D_OUT), dtype=np.float32)
    for core in range(8):
        cn, co = divmod(core, MESH_O)
        y[cn * N_SHARD:(cn + 1) * N_SHARD, co * O_SHARD:(co + 1) * O_SHARD] = \
            res.results[core]["y"]
    return y
